# revision 42
# baseline (speedup 1.0000x reference)
"""Trainium2 Bass kernel for nn_Loss_4861902979528.

Computes, for embeddings [N,D] and adj [N,N]:
    e      = embeddings / max(||row||_4, 1e-12)
    log_p  = log(e + 1e-10)
    kl     = p_log_p[:,None] - e @ log_p.T
    adj_n  = adj / max(row_l1(adj), 1e-12)
    out    = lambda * sum(kl * adj_n)

Restructured to avoid materializing any [N,N] intermediate:
    out = lam * ( sum_i plp[i]*rsn[i]  -  sum_{j,d} log_p[j,d] * V[j,d] )
    V[j,d] = sum_i adj[i,j] * recip[i] * e[i,d]     (PE matmul, adj streamed once)
    rsn[i] = rowsum(adj)[i] * recip[i]
    recip[i] = 1/max(rowsum(adj)[i], 1e-12)

Sharding: row-blocks of adj across 8 cores (each core owns N/8 = 1536 rows).
Each core computes partial scalars; host sums the partials (scalar all-reduce
done on host) and applies lambda.
"""

import numpy as np

import concourse.bass as bass
import concourse.tile as tile
import concourse.mybir as mybir
from concourse.bass_utils import run_bass_kernel_spmd
from concourse.masks import make_identity

N = 12288          # rows/cols of adj; rows of embeddings
D = 128            # embedding dim
NCORES = 8
R = N // NCORES    # 1536 rows of adj per core
NCH = R // 128     # 12 own 128-row chunks per core
NFULL = N // 128   # 96 chunks in full embeddings
JW = 512           # j-window (one PSUM bank of fp32)
NJW = N // JW      # 24 windows
SB = 2             # adj row-blocks per super-block (PSUM accumulation depth)
NSB = NCH // SB    # 6 super-blocks
EPS_NORM = 1e-12
EPS_LOG = 1e-10

F32 = mybir.dt.float32
F32R = mybir.dt.float32r
BF16 = mybir.dt.bfloat16
AF = mybir.ActivationFunctionType
ALU = mybir.AluOpType

# The V-matmul runs in bf16: the ACT pass that computes row sums doubles as
# a cast into a separate bf16 half-block pool, so the f32 landing buffer is
# freed for the next DMA as soon as the conversion pass is done. bf16
# streams through the PE at 1 cyc/row (fp32 would be 4) and the contraction
# over 1536 rows averages the rounding noise to ~1e-5 relative on the final
# scalar.

# Fused multiply+reduce on DVE. "stt" uses scalar_tensor_tensor with
# accum_out (1 DVE op); "2op" falls back to tensor_mul + reduce_sum.
MUL_REDUCE_MODE = "stt"

# Route half-1 cast+rowsum to GPSIMD (load-balance vs ACT).
CONVERT_SPLIT = False
SOLO_TAIL = False


def _mul_reduce(nc, out_scratch, in0, in1, accum_col):
    if MUL_REDUCE_MODE == "stt":
        nc.vector.scalar_tensor_tensor(
            out=out_scratch, in0=in0, scalar=1.0, in1=in1,
            op0=ALU.mult, op1=ALU.mult, accum_out=accum_col,
        )
    else:
        nc.vector.tensor_mul(out_scratch, in0, in1)
        nc.vector.reduce_sum(accum_col, out_scratch, axis=mybir.AxisListType.X)


def _split_excess_waits(nc: bass.Bass, max_waits: int = 1) -> None:
    """This walrus build rejects instructions carrying more than a couple of
    semaphore waits ("Too many sync wait commands"). Hoist excess waits onto
    same-engine NOPs inserted just before the offending instruction."""
    n_split = 0
    for fn in nc.m.functions:
        for bb in fn.blocks:
            insts = bb.instructions
            out = []
            changed = False
            for inst in insts:
                si = inst.sync_info
                waits = list(si.on_wait) if si is not None and si.on_wait else []
                if len(waits) > max_waits:
                    extra, keep = waits[:-max_waits], waits[-max_waits:]
                    for i in range(0, len(extra), max_waits):
                        n_split += 1
                        out.append(
                            mybir.InstNoOp(
                                name=f"{inst.name}-ws{i}",
                                engine=inst.engine,
                                sync_info=mybir.SyncInfo(
                                    on_wait=extra[i : i + max_waits], on_update=[]
                                ),
                                bass_nofuse=True,
                            )
                        )
                    inst.sync_info = mybir.SyncInfo(
                        on_wait=keep,
                        on_update=list(si.on_update) if si.on_update else [],
                    )
                    changed = True
                out.append(inst)
            if changed:
                bb.instructions = out


def build_program() -> bass.Bass:
    nc = bass.Bass()

    adj = nc.declare_dram_parameter("adj_block", [R, N], F32, isOutput=False)
    emb = nc.declare_dram_parameter("emb", [N, D], F32, isOutput=False)
    emb_own = nc.declare_dram_parameter("emb_own", [R, D], F32, isOutput=False)
    acc1_d = nc.declare_dram_parameter("acc1", [128, NCH], F32, isOutput=True)
    acc2_d = nc.declare_dram_parameter("acc2", [128, (NSB + 1) * NJW], F32, isOutput=True)

    with tile.TileContext(nc) as tc:
        with (
            tc.tile_pool(name="blk", bufs=2) as blk_pool,
            tc.tile_pool(name="bfb", bufs=5) as bf_pool,
            tc.tile_pool(name="persist", bufs=1) as persist,
            tc.tile_pool(name="chunk", bufs=3) as chunk_pool,
            tc.tile_pool(name="sq", bufs=2) as sq_pool,
            tc.tile_pool(name="trash", bufs=2) as trash_pool,
            tc.tile_pool(name="e2", bufs=3) as e2_pool,
            tc.tile_pool(name="psum_v", bufs=6, space="PSUM") as psum_v,
            tc.tile_pool(name="psum_t", bufs=2, space="PSUM") as psum_t,
        ):
            # ---- persistent small tiles ----
            logpT = persist.tile([128, N], BF16, tag="logpT")   # log_p transposed [d, j]
            e_own = persist.tile([128, R], F32, tag="e_own")    # normalized e, own rows
            embo = persist.tile([128, NCH, D], F32, tag="embo")
            ident = persist.tile([128, 128], F32, tag="ident")
            s4f = persist.tile([128, NFULL], F32, tag="s4f")
            rnf = persist.tile([128, NFULL], F32, tag="rnf")
            s4o = persist.tile([128, NCH], F32, tag="s4o")
            rno = persist.tile([128, NCH], F32, tag="rno")
            plp = persist.tile([128, NCH], F32, tag="plp")      # sum_d e*log_p per own row
            rsh = persist.tile([128, 4 * NCH], F32, tag="rsh")  # per-piece row sums
            rs = persist.tile([128, NCH], F32, tag="rs")        # adj row sums
            rsc = persist.tile([128, NCH], F32, tag="rsc")      # max(rs, eps)
            rc = persist.tile([128, NCH], F32, tag="rc")        # 1/max(rs, eps)
            acc1_t = persist.tile([128, NCH], F32, tag="acc1")
            acc2_t = persist.tile([128, (NSB + 1) * NJW], F32, tag="acc2")
            epsl = persist.tile([128, 1], F32, tag="epsl")

            make_identity(nc, ident)
            nc.vector.memset(epsl, EPS_LOG)

            # ================= prep: own rows (plp, e_own) =================
            nc.sync.dma_start(
                out=embo, in_=emb_own.rearrange("(n p) d -> p n d", p=128)
            )
            for oc in range(NCH):
                src = embo[:, oc, :]
                sq = sq_pool.tile([128, D], F32)
                nc.gpsimd.tensor_mul(sq, src, src)
                tr = trash_pool.tile([128, D], F32)
                _mul_reduce(nc, tr, sq, sq, s4o[:, oc : oc + 1])
            # rnorm = (max(s4, eps^4))^(-1/4) via exp(-ln(s4)/4)
            nc.vector.tensor_scalar_max(s4o, s4o, EPS_NORM**4)
            nc.scalar.activation(rno, s4o, AF.Ln)
            nc.scalar.activation(rno, rno, AF.Exp, scale=-0.25)
            for oc in range(NCH):
                esl = e_own[:, oc * 128 : (oc + 1) * 128]
                nc.vector.tensor_scalar_mul(esl, embo[:, oc, :], rno[:, oc : oc + 1])
                lg = chunk_pool.tile([128, D], F32)
                nc.scalar.activation(lg, esl, AF.Ln, bias=epsl)
                tr = trash_pool.tile([128, D], F32)
                _mul_reduce(nc, tr, esl, lg, plp[:, oc : oc + 1])

            # ================= prep: full embeddings -> logpT =================
            # One streaming pass in quarter-buffers sized like the adj
            # half-blocks (shared pool slots). The 4-norm is per-row, so
            # each quarter runs its full chain (s4 -> rnorm -> normalize ->
            # transpose -> log) while its staging tile is still resident.
            NQ = 4
            CPQ = NFULL // NQ  # chunks per quarter
            emb_v = emb.rearrange("(n p) d -> p n d", p=128)
            for q in range(NQ):
                embq = bf_pool.tile([128, CPQ, D], F32, tag="bfb")
                nc.sync.dma_start(
                    out=embq, in_=emb_v[:, q * CPQ : (q + 1) * CPQ, :]
                )
                for c in range(CPQ):
                    n = q * CPQ + c
                    src = embq[:, c, :]
                    sq = sq_pool.tile([128, D], F32)
                    nc.gpsimd.tensor_mul(sq, src, src)
                    tr = trash_pool.tile([128, D], F32)
                    _mul_reduce(nc, tr, sq, sq, s4f[:, n : n + 1])
                qs = slice(q * CPQ, (q + 1) * CPQ)
                nc.vector.tensor_scalar_max(s4f[:, qs], s4f[:, qs], EPS_NORM**4)
                nc.scalar.activation(rnf[:, qs], s4f[:, qs], AF.Ln)
                nc.scalar.activation(rnf[:, qs], rnf[:, qs], AF.Exp, scale=-0.25)
                for c in range(CPQ):
                    n = q * CPQ + c
                    ec = chunk_pool.tile([128, 128], F32)
                    nc.vector.tensor_scalar_mul(
                        ec, embq[:, c, :], rnf[:, n : n + 1]
                    )
                    pt = psum_t.tile([128, 128], F32)
                    nc.tensor.transpose(pt, ec, ident)
                    nc.scalar.activation(
                        logpT[:, n * 128 : (n + 1) * 128], pt, AF.Ln, bias=epsl
                    )

            # ================= hot loop: stream adj row blocks =================
            # Each 128-row adj block is loaded as NP column pieces
            # [128, N/NP]; the ACT pass converts each piece to bf16 (separate
            # pool) and accumulates per-piece row sums. The jw loop consumes
            # piece p of both blocks before piece p+1, so piece slots retire
            # as the phase progresses and the tail drain is short.
            def do_group(ics, np_pieces, acc_base, gpsimd_conv_pieces=()):
                """Process a group of 128-row adj blocks: DMA pieces, cast+
                rowsum, then matmul-accumulate the group into PSUM windows
                and fold each window into acc2 columns."""
                npw = N // np_pieces
                njp = NJW // np_pieces
                pieces = []
                e2s = []
                for k, ic in enumerate(ics):
                    kp = []
                    for p in range(np_pieces):
                        blk = blk_pool.tile([128, npw], F32, tag="blk")
                        nc.sync.dma_start(
                            out=blk,
                            in_=adj[
                                ic * 128 : (ic + 1) * 128,
                                p * npw : (p + 1) * npw,
                            ],
                        )
                        bfb = bf_pool.tile([128, npw], BF16, tag="bfb")
                        rcol = rsh[:, 4 * ic + p : 4 * ic + p + 1]
                        if (k, p) in gpsimd_conv_pieces:
                            nc.gpsimd.tensor_scalar(
                                out=bfb, in0=blk, scalar1=0.0, scalar2=None,
                                op0=ALU.add, accum_out=rcol,
                            )
                        else:
                            nc.scalar.activation(
                                bfb, blk, AF.Copy, accum_out=rcol
                            )
                        kp.append(bfb)
                    pieces.append(kp)
                    nc.vector.reduce_sum(
                        rs[:, ic : ic + 1],
                        rsh[:, 4 * ic : 4 * ic + np_pieces],
                        axis=mybir.AxisListType.X,
                    )
                    nc.vector.tensor_scalar_max(
                        rsc[:, ic : ic + 1], rs[:, ic : ic + 1], EPS_NORM
                    )
                    nc.vector.reciprocal(rc[:, ic : ic + 1], rsc[:, ic : ic + 1])
                    e2 = e2_pool.tile([128, 128], BF16)
                    nc.vector.tensor_scalar_mul(
                        e2, e_own[:, ic * 128 : (ic + 1) * 128], rc[:, ic : ic + 1]
                    )
                    e2s.append(e2)
                for jw in range(NJW):
                    p, jl = jw // njp, jw % njp
                    pv = psum_v.tile([128, JW], F32)
                    for k in range(len(ics)):
                        nc.tensor.matmul(
                            pv,
                            e2s[k],
                            pieces[k][p][:, jl * JW : (jl + 1) * JW],
                            start=(k == 0),
                            stop=(k == len(ics) - 1),
                        )
                    _mul_reduce(
                        nc, pv, pv, logpT[:, jw * JW : (jw + 1) * JW],
                        acc2_t[:, acc_base + jw : acc_base + jw + 1],
                    )

            # Paired blocks with column halves for most of the stream; the
            # ACT cast of the second half runs on GPSIMD for the first few
            # groups where ACT is also busy building logpT. The last two
            # blocks run solo with column quarters so the pipeline tail
            # after the final DMA is short.
            if SOLO_TAIL:
                for sb in range(NSB - 1):
                    do_group([sb * SB, sb * SB + 1], 2, sb * NJW)
                do_group([R // 128 - 2], 4, (NSB - 1) * NJW)
                do_group([R // 128 - 1], 4, NSB * NJW)
            else:
                for sb in range(NSB):
                    do_group([sb * SB, sb * SB + 1], 2, sb * NJW)

            # term1 = plp * rs * rc  (rs*rc = rowsum of normalized adj, ~= 1)
            nc.vector.tensor_mul(acc1_t, rs, rc)
            nc.vector.tensor_mul(acc1_t, acc1_t, plp)
            nc.sync.dma_start(out=acc1_d[:, :], in_=acc1_t)

            nc.sync.dma_start(out=acc2_d[:, :], in_=acc2_t)

    _split_excess_waits(nc)
    return nc


_PROGRAM = None


def _get_program():
    global _PROGRAM
    if _PROGRAM is None:
        _PROGRAM = build_program()
    return _PROGRAM


def kernel(embeddings: np.ndarray, adj: np.ndarray, lambda_reg: np.ndarray) -> np.ndarray:
    embeddings = np.ascontiguousarray(np.asarray(embeddings, dtype=np.float32))
    adj = np.asarray(adj, dtype=np.float32)

    nc = _get_program()
    in_maps = []
    for c in range(NCORES):
        in_maps.append(
            {
                "adj_block": np.ascontiguousarray(adj[c * R : (c + 1) * R, :]),
                "emb": embeddings,
                "emb_own": np.ascontiguousarray(embeddings[c * R : (c + 1) * R, :]),
            }
        )
    out = run_bass_kernel_spmd(nc, in_maps, core_ids=list(range(NCORES)))
    total = np.float64(0.0)
    for r in out.results:
        total += r["acc1"].sum(dtype=np.float64) - r["acc2"].sum(dtype=np.float64)
    lam = np.float64(np.asarray(lambda_reg, dtype=np.float32))
    return np.asarray(np.float32(lam * total))


# revision 45
# speedup vs baseline: 9.3687x; 9.3687x over previous
"""Trainium2 Bass kernel for nn_Loss_4861902979528.

Computes, for embeddings [N,D] and adj [N,N]:
    e      = embeddings / max(||row||_4, 1e-12)
    log_p  = log(e + 1e-10)
    kl     = p_log_p[:,None] - e @ log_p.T
    adj_n  = adj / max(row_l1(adj), 1e-12)
    out    = lambda * sum(kl * adj_n)

Restructured to avoid materializing any [N,N] intermediate:
    out = lam * ( sum_i plp[i]*rsn[i]  -  sum_{j,d} log_p[j,d] * V[j,d] )
    V[j,d] = sum_i adj[i,j] * recip[i] * e[i,d]     (PE matmul, adj streamed once)
    rsn[i] = rowsum(adj)[i] * recip[i]
    recip[i] = 1/max(rowsum(adj)[i], 1e-12)

Sharding: row-blocks of adj across 8 cores (each core owns N/8 = 1536 rows).
Each core computes partial scalars; host sums the partials (scalar all-reduce
done on host) and applies lambda.
"""

import numpy as np

import concourse.bass as bass
import concourse.tile as tile
import concourse.mybir as mybir
from concourse.bass_utils import run_bass_kernel_spmd
from concourse.masks import make_identity

N = 12288          # rows/cols of adj; rows of embeddings
D = 128            # embedding dim
NCORES = 8
R = N // NCORES    # 1536 rows of adj per core
NCH = R // 128     # 12 own 128-row chunks per core
NFULL = N // 128   # 96 chunks in full embeddings
JW = 512           # j-window (one PSUM bank of fp32)
NJW = N // JW      # 24 windows
SB = 2             # adj row-blocks per super-block (PSUM accumulation depth)
NSB = NCH // SB    # 6 super-blocks
EPS_NORM = 1e-12
EPS_LOG = 1e-10

F32 = mybir.dt.float32
F32R = mybir.dt.float32r
BF16 = mybir.dt.bfloat16
AF = mybir.ActivationFunctionType
ALU = mybir.AluOpType

# The V-matmul runs in bf16: the ACT pass that computes row sums doubles as
# a cast into a separate bf16 half-block pool, so the f32 landing buffer is
# freed for the next DMA as soon as the conversion pass is done. bf16
# streams through the PE at 1 cyc/row (fp32 would be 4) and the contraction
# over 1536 rows averages the rounding noise to ~1e-5 relative on the final
# scalar.

# Fused multiply+reduce on DVE. "stt" uses scalar_tensor_tensor with
# accum_out (1 DVE op); "2op" falls back to tensor_mul + reduce_sum.
MUL_REDUCE_MODE = "stt"

# Route half-1 cast+rowsum to GPSIMD (load-balance vs ACT).
CONVERT_SPLIT = False
SOLO_TAIL = False


def _mul_reduce(nc, out_scratch, in0, in1, accum_col):
    if MUL_REDUCE_MODE == "stt":
        nc.vector.scalar_tensor_tensor(
            out=out_scratch, in0=in0, scalar=1.0, in1=in1,
            op0=ALU.mult, op1=ALU.mult, accum_out=accum_col,
        )
    else:
        nc.vector.tensor_mul(out_scratch, in0, in1)
        nc.vector.reduce_sum(accum_col, out_scratch, axis=mybir.AxisListType.X)


def _split_excess_waits(nc: bass.Bass, max_waits: int = 1) -> None:
    """This walrus build rejects instructions carrying more than a couple of
    semaphore waits ("Too many sync wait commands"). Hoist excess waits onto
    same-engine NOPs inserted just before the offending instruction."""
    n_split = 0
    for fn in nc.m.functions:
        for bb in fn.blocks:
            insts = bb.instructions
            out = []
            changed = False
            for inst in insts:
                si = inst.sync_info
                waits = list(si.on_wait) if si is not None and si.on_wait else []
                if len(waits) > max_waits:
                    extra, keep = waits[:-max_waits], waits[-max_waits:]
                    for i in range(0, len(extra), max_waits):
                        n_split += 1
                        out.append(
                            mybir.InstNoOp(
                                name=f"{inst.name}-ws{i}",
                                engine=inst.engine,
                                sync_info=mybir.SyncInfo(
                                    on_wait=extra[i : i + max_waits], on_update=[]
                                ),
                                bass_nofuse=True,
                            )
                        )
                    inst.sync_info = mybir.SyncInfo(
                        on_wait=keep,
                        on_update=list(si.on_update) if si.on_update else [],
                    )
                    changed = True
                out.append(inst)
            if changed:
                bb.instructions = out


def build_program() -> bass.Bass:
    nc = bass.Bass()

    adj = nc.declare_dram_parameter("adj_block", [R, N], F32, isOutput=False)
    emb = nc.declare_dram_parameter("emb", [N, D], F32, isOutput=False)
    emb_own = nc.declare_dram_parameter("emb_own", [R, D], F32, isOutput=False)
    acc1_d = nc.declare_dram_parameter("acc1", [128, NCH], F32, isOutput=True)
    acc2_d = nc.declare_dram_parameter("acc2", [128, (NSB + 1) * NJW], F32, isOutput=True)

    with tile.TileContext(nc) as tc:
        with (
            tc.tile_pool(name="blk", bufs=2) as blk_pool,
            tc.tile_pool(name="bfb", bufs=5) as bf_pool,
            tc.tile_pool(name="persist", bufs=1) as persist,
            tc.tile_pool(name="chunk", bufs=3) as chunk_pool,
            tc.tile_pool(name="sq", bufs=2) as sq_pool,
            tc.tile_pool(name="trash", bufs=2) as trash_pool,
            tc.tile_pool(name="e2", bufs=3) as e2_pool,
            tc.tile_pool(name="psum_v", bufs=6, space="PSUM") as psum_v,
            tc.tile_pool(name="psum_t", bufs=2, space="PSUM") as psum_t,
        ):
            # ---- persistent small tiles ----
            logpT = persist.tile([128, N], BF16, tag="logpT")   # log_p transposed [d, j]
            e_own = persist.tile([128, R], F32, tag="e_own")    # normalized e, own rows
            embo = persist.tile([128, NCH, D], F32, tag="embo")
            ident = persist.tile([128, 128], F32, tag="ident")
            s4f = persist.tile([128, NFULL], F32, tag="s4f")
            rnf = persist.tile([128, NFULL], F32, tag="rnf")
            s4o = persist.tile([128, NCH], F32, tag="s4o")
            rno = persist.tile([128, NCH], F32, tag="rno")
            plp = persist.tile([128, NCH], F32, tag="plp")      # sum_d e*log_p per own row
            rsh = persist.tile([128, 4 * NCH], F32, tag="rsh")  # per-piece row sums
            rs = persist.tile([128, NCH], F32, tag="rs")        # adj row sums
            rsc = persist.tile([128, NCH], F32, tag="rsc")      # max(rs, eps)
            rc = persist.tile([128, NCH], F32, tag="rc")        # 1/max(rs, eps)
            acc1_t = persist.tile([128, NCH], F32, tag="acc1")
            acc2_t = persist.tile([128, (NSB + 1) * NJW], F32, tag="acc2")
            epsl = persist.tile([128, 1], F32, tag="epsl")

            make_identity(nc, ident)
            nc.vector.memset(epsl, EPS_LOG)

            # ================= prep: own rows (plp, e_own) =================
            nc.sync.dma_start(
                out=embo, in_=emb_own.rearrange("(n p) d -> p n d", p=128)
            )
            for oc in range(NCH):
                src = embo[:, oc, :]
                sq = sq_pool.tile([128, D], F32)
                nc.gpsimd.tensor_mul(sq, src, src)
                tr = trash_pool.tile([128, D], F32)
                _mul_reduce(nc, tr, sq, sq, s4o[:, oc : oc + 1])
            # rnorm = (max(s4, eps^4))^(-1/4) via exp(-ln(s4)/4)
            nc.vector.tensor_scalar_max(s4o, s4o, EPS_NORM**4)
            nc.scalar.activation(rno, s4o, AF.Ln)
            nc.scalar.activation(rno, rno, AF.Exp, scale=-0.25)
            for oc in range(NCH):
                esl = e_own[:, oc * 128 : (oc + 1) * 128]
                nc.vector.tensor_scalar_mul(esl, embo[:, oc, :], rno[:, oc : oc + 1])
                lg = chunk_pool.tile([128, D], F32)
                nc.scalar.activation(lg, esl, AF.Ln, bias=epsl)
                tr = trash_pool.tile([128, D], F32)
                _mul_reduce(nc, tr, esl, lg, plp[:, oc : oc + 1])

            # ================= prep: full embeddings -> logpT =================
            # One streaming pass in quarter-buffers sized like the adj
            # half-blocks (shared pool slots). The 4-norm is per-row, so
            # each quarter runs its full chain (s4 -> rnorm -> normalize ->
            # transpose -> log) while its staging tile is still resident.
            NQ = 4
            CPQ = NFULL // NQ  # chunks per quarter
            emb_v = emb.rearrange("(n p) d -> p n d", p=128)
            for q in range(NQ):
                embq = bf_pool.tile([128, CPQ, D], F32, tag="bfb")
                nc.sync.dma_start(
                    out=embq, in_=emb_v[:, q * CPQ : (q + 1) * CPQ, :]
                )
                for c in range(CPQ):
                    n = q * CPQ + c
                    src = embq[:, c, :]
                    sq = sq_pool.tile([128, D], F32)
                    if c % 2 == 0:
                        nc.gpsimd.tensor_mul(sq, src, src)
                    else:
                        nc.vector.tensor_mul(sq, src, src)
                    tr = trash_pool.tile([128, D], F32)
                    _mul_reduce(nc, tr, sq, sq, s4f[:, n : n + 1])
                qs = slice(q * CPQ, (q + 1) * CPQ)
                nc.vector.tensor_scalar_max(s4f[:, qs], s4f[:, qs], EPS_NORM**4)
                nc.scalar.activation(rnf[:, qs], s4f[:, qs], AF.Ln)
                nc.scalar.activation(rnf[:, qs], rnf[:, qs], AF.Exp, scale=-0.25)
                for c2 in range(CPQ // 2):
                    n0 = q * CPQ + 2 * c2
                    pt = psum_t.tile([128, 256], F32)
                    for u in range(2):
                        n = n0 + u
                        ec = chunk_pool.tile([128, 128], F32)
                        nc.vector.tensor_scalar_mul(
                            ec, embq[:, 2 * c2 + u, :], rnf[:, n : n + 1]
                        )
                        nc.tensor.transpose(pt[:, u * 128 : (u + 1) * 128], ec, ident)
                    nc.scalar.activation(
                        logpT[:, n0 * 128 : (n0 + 2) * 128], pt, AF.Ln, bias=epsl
                    )

            # ================= hot loop: stream adj row blocks =================
            # Each 128-row adj block is loaded as NP column pieces
            # [128, N/NP]; the ACT pass converts each piece to bf16 (separate
            # pool) and accumulates per-piece row sums. The jw loop consumes
            # piece p of both blocks before piece p+1, so piece slots retire
            # as the phase progresses and the tail drain is short.
            def do_group(ics, np_pieces, acc_base, gpsimd_conv_pieces=()):
                """Process a group of 128-row adj blocks: DMA pieces, cast+
                rowsum, then matmul-accumulate the group into PSUM windows
                and fold each window into acc2 columns."""
                npw = N // np_pieces
                njp = NJW // np_pieces
                pieces = []
                e2s = []
                for k, ic in enumerate(ics):
                    kp = []
                    for p in range(np_pieces):
                        blk = blk_pool.tile([128, npw], F32, tag="blk")
                        nc.sync.dma_start(
                            out=blk,
                            in_=adj[
                                ic * 128 : (ic + 1) * 128,
                                p * npw : (p + 1) * npw,
                            ],
                        )
                        bfb = bf_pool.tile([128, npw], BF16, tag="bfb")
                        rcol = rsh[:, 4 * ic + p : 4 * ic + p + 1]
                        if (k, p) in gpsimd_conv_pieces:
                            nc.gpsimd.tensor_scalar(
                                out=bfb, in0=blk, scalar1=0.0, scalar2=None,
                                op0=ALU.add, accum_out=rcol,
                            )
                        else:
                            nc.scalar.activation(
                                bfb, blk, AF.Copy, accum_out=rcol
                            )
                        kp.append(bfb)
                    pieces.append(kp)
                    nc.vector.reduce_sum(
                        rs[:, ic : ic + 1],
                        rsh[:, 4 * ic : 4 * ic + np_pieces],
                        axis=mybir.AxisListType.X,
                    )
                    nc.vector.tensor_scalar_max(
                        rsc[:, ic : ic + 1], rs[:, ic : ic + 1], EPS_NORM
                    )
                    nc.vector.reciprocal(rc[:, ic : ic + 1], rsc[:, ic : ic + 1])
                    e2 = e2_pool.tile([128, 128], BF16)
                    nc.vector.tensor_scalar_mul(
                        e2, e_own[:, ic * 128 : (ic + 1) * 128], rc[:, ic : ic + 1]
                    )
                    e2s.append(e2)
                for jw in range(NJW):
                    p, jl = jw // njp, jw % njp
                    pv = psum_v.tile([128, JW], F32)
                    for k in range(len(ics)):
                        nc.tensor.matmul(
                            pv,
                            e2s[k],
                            pieces[k][p][:, jl * JW : (jl + 1) * JW],
                            start=(k == 0),
                            stop=(k == len(ics) - 1),
                        )
                    _mul_reduce(
                        nc, pv, pv, logpT[:, jw * JW : (jw + 1) * JW],
                        acc2_t[:, acc_base + jw : acc_base + jw + 1],
                    )

            # Paired blocks with column halves for most of the stream; the
            # ACT cast of the second half runs on GPSIMD for the first few
            # groups where ACT is also busy building logpT. The last two
            # blocks run solo with column quarters so the pipeline tail
            # after the final DMA is short.
            if SOLO_TAIL:
                for sb in range(NSB - 1):
                    do_group([sb * SB, sb * SB + 1], 2, sb * NJW)
                do_group([R // 128 - 2], 2, (NSB - 1) * NJW)
                do_group([R // 128 - 1], 2, NSB * NJW)
            else:
                for sb in range(NSB):
                    do_group([sb * SB, sb * SB + 1], 2, sb * NJW)

            # term1 = plp * rs * rc  (rs*rc = rowsum of normalized adj, ~= 1)
            nc.vector.tensor_mul(acc1_t, rs, rc)
            nc.vector.tensor_mul(acc1_t, acc1_t, plp)
            nc.sync.dma_start(out=acc1_d[:, :], in_=acc1_t)

            nc.sync.dma_start(out=acc2_d[:, :], in_=acc2_t)

    _split_excess_waits(nc)
    return nc


_PROGRAM = None


def _get_program():
    global _PROGRAM
    if _PROGRAM is None:
        _PROGRAM = build_program()
    return _PROGRAM


def kernel(embeddings: np.ndarray, adj: np.ndarray, lambda_reg: np.ndarray) -> np.ndarray:
    embeddings = np.ascontiguousarray(np.asarray(embeddings, dtype=np.float32))
    adj = np.asarray(adj, dtype=np.float32)

    nc = _get_program()
    in_maps = []
    for c in range(NCORES):
        in_maps.append(
            {
                "adj_block": np.ascontiguousarray(adj[c * R : (c + 1) * R, :]),
                "emb": embeddings,
                "emb_own": np.ascontiguousarray(embeddings[c * R : (c + 1) * R, :]),
            }
        )
    out = run_bass_kernel_spmd(nc, in_maps, core_ids=list(range(NCORES)))
    total = np.float64(0.0)
    for r in out.results:
        total += r["acc1"].sum(dtype=np.float64) - r["acc2"].sum(dtype=np.float64)
    lam = np.float64(np.asarray(lambda_reg, dtype=np.float32))
    return np.asarray(np.float32(lam * total))


# revision 48
# speedup vs baseline: 9.5271x; 1.0169x over previous
"""Trainium2 Bass kernel for nn_Loss_4861902979528.

Computes, for embeddings [N,D] and adj [N,N]:
    e      = embeddings / max(||row||_4, 1e-12)
    log_p  = log(e + 1e-10)
    kl     = p_log_p[:,None] - e @ log_p.T
    adj_n  = adj / max(row_l1(adj), 1e-12)
    out    = lambda * sum(kl * adj_n)

Restructured to avoid materializing any [N,N] intermediate:
    out = lam * ( sum_i plp[i]*rsn[i]  -  sum_{j,d} log_p[j,d] * V[j,d] )
    V[j,d] = sum_i adj[i,j] * recip[i] * e[i,d]     (PE matmul, adj streamed once)
    rsn[i] = rowsum(adj)[i] * recip[i]
    recip[i] = 1/max(rowsum(adj)[i], 1e-12)

Sharding: row-blocks of adj across 8 cores (each core owns N/8 = 1536 rows).
Each core computes partial scalars; host sums the partials (scalar all-reduce
done on host) and applies lambda.
"""

import numpy as np

import concourse.bass as bass
import concourse.tile as tile
import concourse.mybir as mybir
from concourse.bass_utils import run_bass_kernel_spmd
from concourse.masks import make_identity

N = 12288          # rows/cols of adj; rows of embeddings
D = 128            # embedding dim
NCORES = 8
R = N // NCORES    # 1536 rows of adj per core
NCH = R // 128     # 12 own 128-row chunks per core
NFULL = N // 128   # 96 chunks in full embeddings
JW = 512           # j-window (one PSUM bank of fp32)
NJW = N // JW      # 24 windows
SB = 2             # adj row-blocks per super-block (PSUM accumulation depth)
NSB = NCH // SB    # 6 super-blocks
EPS_NORM = 1e-12
EPS_LOG = 1e-10

F32 = mybir.dt.float32
F32R = mybir.dt.float32r
BF16 = mybir.dt.bfloat16
AF = mybir.ActivationFunctionType
ALU = mybir.AluOpType

# The V-matmul runs in bf16: the ACT pass that computes row sums doubles as
# a cast into a separate bf16 half-block pool, so the f32 landing buffer is
# freed for the next DMA as soon as the conversion pass is done. bf16
# streams through the PE at 1 cyc/row (fp32 would be 4) and the contraction
# over 1536 rows averages the rounding noise to ~1e-5 relative on the final
# scalar.

# Fused multiply+reduce on DVE. "stt" uses scalar_tensor_tensor with
# accum_out (1 DVE op); "2op" falls back to tensor_mul + reduce_sum.
MUL_REDUCE_MODE = "stt"

# Route half-1 cast+rowsum to GPSIMD (load-balance vs ACT).
CONVERT_SPLIT = False
SOLO_TAIL = False


def _mul_reduce(nc, out_scratch, in0, in1, accum_col):
    if MUL_REDUCE_MODE == "stt":
        nc.vector.scalar_tensor_tensor(
            out=out_scratch, in0=in0, scalar=1.0, in1=in1,
            op0=ALU.mult, op1=ALU.mult, accum_out=accum_col,
        )
    else:
        nc.vector.tensor_mul(out_scratch, in0, in1)
        nc.vector.reduce_sum(accum_col, out_scratch, axis=mybir.AxisListType.X)


def _split_excess_waits(nc: bass.Bass, max_waits: int = 1) -> None:
    """This walrus build rejects instructions carrying more than a couple of
    semaphore waits ("Too many sync wait commands"). Hoist excess waits onto
    same-engine NOPs inserted just before the offending instruction."""
    n_split = 0
    for fn in nc.m.functions:
        for bb in fn.blocks:
            insts = bb.instructions
            out = []
            changed = False
            for inst in insts:
                si = inst.sync_info
                waits = list(si.on_wait) if si is not None and si.on_wait else []
                if len(waits) > max_waits:
                    extra, keep = waits[:-max_waits], waits[-max_waits:]
                    for i in range(0, len(extra), max_waits):
                        n_split += 1
                        out.append(
                            mybir.InstNoOp(
                                name=f"{inst.name}-ws{i}",
                                engine=inst.engine,
                                sync_info=mybir.SyncInfo(
                                    on_wait=extra[i : i + max_waits], on_update=[]
                                ),
                                bass_nofuse=True,
                            )
                        )
                    inst.sync_info = mybir.SyncInfo(
                        on_wait=keep,
                        on_update=list(si.on_update) if si.on_update else [],
                    )
                    changed = True
                out.append(inst)
            if changed:
                bb.instructions = out


def build_program() -> bass.Bass:
    nc = bass.Bass()

    adj = nc.declare_dram_parameter("adj_block", [R, N], F32, isOutput=False)
    emb = nc.declare_dram_parameter("emb", [N, D], F32, isOutput=False)
    emb_own = nc.declare_dram_parameter("emb_own", [R, D], F32, isOutput=False)
    acc1_d = nc.declare_dram_parameter("acc1", [128, NCH], F32, isOutput=True)
    acc2_d = nc.declare_dram_parameter("acc2", [128, (NSB + 1) * NJW], F32, isOutput=True)

    with tile.TileContext(nc) as tc:
        with (
            tc.tile_pool(name="blk", bufs=2) as blk_pool,
            tc.tile_pool(name="bfb", bufs=6) as bf_pool,
            tc.tile_pool(name="persist", bufs=1) as persist,
            tc.tile_pool(name="chunk", bufs=3) as chunk_pool,
            tc.tile_pool(name="sq", bufs=2) as sq_pool,
            tc.tile_pool(name="trash", bufs=2) as trash_pool,
            tc.tile_pool(name="e2", bufs=3) as e2_pool,
            tc.tile_pool(name="psum_v", bufs=6, space="PSUM") as psum_v,
            tc.tile_pool(name="psum_t", bufs=2, space="PSUM") as psum_t,
        ):
            # ---- persistent small tiles ----
            logpT = persist.tile([128, N], BF16, tag="logpT")   # log_p transposed [d, j]
            e_own = persist.tile([128, R], F32, tag="e_own")    # normalized e, own rows
            embo = persist.tile([128, NCH, D], F32, tag="embo")
            ident = persist.tile([128, 128], F32, tag="ident")
            s4f = persist.tile([128, NFULL], F32, tag="s4f")
            rnf = persist.tile([128, NFULL], F32, tag="rnf")
            s4o = persist.tile([128, NCH], F32, tag="s4o")
            rno = persist.tile([128, NCH], F32, tag="rno")
            plp = persist.tile([128, NCH], F32, tag="plp")      # sum_d e*log_p per own row
            rsh = persist.tile([128, 4 * NCH], F32, tag="rsh")  # per-piece row sums
            rs = persist.tile([128, NCH], F32, tag="rs")        # adj row sums
            rsc = persist.tile([128, NCH], F32, tag="rsc")      # max(rs, eps)
            rc = persist.tile([128, NCH], F32, tag="rc")        # 1/max(rs, eps)
            acc1_t = persist.tile([128, NCH], F32, tag="acc1")
            acc2_t = persist.tile([128, (NSB + 1) * NJW], F32, tag="acc2")
            epsl = persist.tile([128, 1], F32, tag="epsl")

            make_identity(nc, ident)
            nc.vector.memset(epsl, EPS_LOG)

            # ================= prep: own rows (plp, e_own) =================
            nc.sync.dma_start(
                out=embo, in_=emb_own.rearrange("(n p) d -> p n d", p=128)
            )
            for oc in range(NCH):
                src = embo[:, oc, :]
                sq = sq_pool.tile([128, D], F32)
                nc.gpsimd.tensor_mul(sq, src, src)
                tr = trash_pool.tile([128, D], F32)
                _mul_reduce(nc, tr, sq, sq, s4o[:, oc : oc + 1])
            # rnorm = (max(s4, eps^4))^(-1/4) via exp(-ln(s4)/4)
            nc.vector.tensor_scalar_max(s4o, s4o, EPS_NORM**4)
            nc.scalar.activation(rno, s4o, AF.Ln)
            nc.scalar.activation(rno, rno, AF.Exp, scale=-0.25)
            for oc in range(NCH):
                esl = e_own[:, oc * 128 : (oc + 1) * 128]
                nc.vector.tensor_scalar_mul(esl, embo[:, oc, :], rno[:, oc : oc + 1])
                lg = chunk_pool.tile([128, D], F32)
                nc.scalar.activation(lg, esl, AF.Ln, bias=epsl)
                tr = trash_pool.tile([128, D], F32)
                _mul_reduce(nc, tr, esl, lg, plp[:, oc : oc + 1])

            # ================= prep: full embeddings -> logpT =================
            # One streaming pass in quarter-buffers sized like the adj
            # half-blocks (shared pool slots). The 4-norm is per-row, so
            # each quarter runs its full chain (s4 -> rnorm -> normalize ->
            # transpose -> log) while its staging tile is still resident.
            NQ = 4
            CPQ = NFULL // NQ  # chunks per quarter
            emb_v = emb.rearrange("(n p) d -> p n d", p=128)
            for q in range(NQ):
                embq = bf_pool.tile([128, CPQ, D], F32, tag="bfb")
                nc.sync.dma_start(
                    out=embq, in_=emb_v[:, q * CPQ : (q + 1) * CPQ, :]
                )
                for c in range(CPQ):
                    n = q * CPQ + c
                    src = embq[:, c, :]
                    sq = sq_pool.tile([128, D], F32)
                    if c % 2 == 0:
                        nc.gpsimd.tensor_mul(sq, src, src)
                    else:
                        nc.vector.tensor_mul(sq, src, src)
                    tr = trash_pool.tile([128, D], F32)
                    _mul_reduce(nc, tr, sq, sq, s4f[:, n : n + 1])
                qs = slice(q * CPQ, (q + 1) * CPQ)
                nc.vector.tensor_scalar_max(s4f[:, qs], s4f[:, qs], EPS_NORM**4)
                nc.scalar.activation(rnf[:, qs], s4f[:, qs], AF.Ln)
                nc.scalar.activation(rnf[:, qs], rnf[:, qs], AF.Exp, scale=-0.25)
                for c2 in range(CPQ // 2):
                    n0 = q * CPQ + 2 * c2
                    pt = psum_t.tile([128, 256], F32)
                    for u in range(2):
                        n = n0 + u
                        ec = chunk_pool.tile([128, 128], F32)
                        nc.vector.tensor_scalar_mul(
                            ec, embq[:, 2 * c2 + u, :], rnf[:, n : n + 1]
                        )
                        nc.tensor.transpose(pt[:, u * 128 : (u + 1) * 128], ec, ident)
                    nc.scalar.activation(
                        logpT[:, n0 * 128 : (n0 + 2) * 128], pt, AF.Ln, bias=epsl
                    )

            # ================= hot loop: stream adj row blocks =================
            # Each 128-row adj block is loaded as NP column pieces
            # [128, N/NP]; the ACT pass converts each piece to bf16 (separate
            # pool) and accumulates per-piece row sums. The jw loop consumes
            # piece p of both blocks before piece p+1, so piece slots retire
            # as the phase progresses and the tail drain is short.
            def do_group(ics, np_pieces, acc_base, dve_conv_last=False):
                """Process a group of 128-row adj blocks: DMA pieces, cast+
                rowsum, then matmul-accumulate the group into PSUM windows
                and fold each window into acc2 columns."""
                npw = N // np_pieces
                njp = NJW // np_pieces
                pieces = []
                e2s = []
                for k, ic in enumerate(ics):
                    kp = []
                    for p in range(np_pieces):
                        blk = blk_pool.tile([128, npw], F32, tag="blk")
                        nc.sync.dma_start(
                            out=blk,
                            in_=adj[
                                ic * 128 : (ic + 1) * 128,
                                p * npw : (p + 1) * npw,
                            ],
                        )
                        bfb = bf_pool.tile([128, npw], BF16, tag="bfb")
                        rcol = rsh[:, 4 * ic + p : 4 * ic + p + 1]
                        if dve_conv_last and k == len(ics) - 1 and p == np_pieces - 1:
                            # the final piece's cast+rowsum runs on DVE in
                            # parallel with ACT converting the other block's
                            # last piece, shortening the pipeline tail
                            nc.vector.tensor_scalar(
                                out=bfb, in0=blk, scalar1=0.0, scalar2=None,
                                op0=ALU.add, accum_out=rcol,
                            )
                        else:
                            nc.scalar.activation(
                                bfb, blk, AF.Copy, accum_out=rcol
                            )
                        kp.append(bfb)
                    pieces.append(kp)
                    nc.vector.reduce_sum(
                        rs[:, ic : ic + 1],
                        rsh[:, 4 * ic : 4 * ic + np_pieces],
                        axis=mybir.AxisListType.X,
                    )
                    nc.vector.tensor_scalar_max(
                        rsc[:, ic : ic + 1], rs[:, ic : ic + 1], EPS_NORM
                    )
                    nc.vector.reciprocal(rc[:, ic : ic + 1], rsc[:, ic : ic + 1])
                    e2 = e2_pool.tile([128, 128], BF16)
                    nc.vector.tensor_scalar_mul(
                        e2, e_own[:, ic * 128 : (ic + 1) * 128], rc[:, ic : ic + 1]
                    )
                    e2s.append(e2)
                for jw in range(NJW):
                    p, jl = jw // njp, jw % njp
                    pv = psum_v.tile([128, JW], F32)
                    for k in range(len(ics)):
                        nc.tensor.matmul(
                            pv,
                            e2s[k],
                            pieces[k][p][:, jl * JW : (jl + 1) * JW],
                            start=(k == 0),
                            stop=(k == len(ics) - 1),
                        )
                    _mul_reduce(
                        nc, pv, pv, logpT[:, jw * JW : (jw + 1) * JW],
                        acc2_t[:, acc_base + jw : acc_base + jw + 1],
                    )

            # Paired blocks with column halves for most of the stream; the
            # ACT cast of the second half runs on GPSIMD for the first few
            # groups where ACT is also busy building logpT. The last two
            # blocks run solo with column quarters so the pipeline tail
            # after the final DMA is short.
            if SOLO_TAIL:
                for sb in range(NSB - 1):
                    do_group([sb * SB, sb * SB + 1], 2, sb * NJW)
                do_group([R // 128 - 2], 2, (NSB - 1) * NJW)
                do_group([R // 128 - 1], 2, NSB * NJW)
            else:
                for sb in range(NSB):
                    do_group([sb * SB, sb * SB + 1], 2, sb * NJW)

            # term1 = plp * rs * rc  (rs*rc = rowsum of normalized adj, ~= 1)
            nc.vector.tensor_mul(acc1_t, rs, rc)
            nc.vector.tensor_mul(acc1_t, acc1_t, plp)
            nc.sync.dma_start(out=acc1_d[:, :], in_=acc1_t)

            nc.sync.dma_start(out=acc2_d[:, :], in_=acc2_t)

    _split_excess_waits(nc)
    return nc


_PROGRAM = None


def _get_program():
    global _PROGRAM
    if _PROGRAM is None:
        _PROGRAM = build_program()
    return _PROGRAM


def kernel(embeddings: np.ndarray, adj: np.ndarray, lambda_reg: np.ndarray) -> np.ndarray:
    embeddings = np.ascontiguousarray(np.asarray(embeddings, dtype=np.float32))
    adj = np.asarray(adj, dtype=np.float32)

    nc = _get_program()
    in_maps = []
    for c in range(NCORES):
        in_maps.append(
            {
                "adj_block": np.ascontiguousarray(adj[c * R : (c + 1) * R, :]),
                "emb": embeddings,
                "emb_own": np.ascontiguousarray(embeddings[c * R : (c + 1) * R, :]),
            }
        )
    out = run_bass_kernel_spmd(nc, in_maps, core_ids=list(range(NCORES)))
    total = np.float64(0.0)
    for r in out.results:
        total += r["acc1"].sum(dtype=np.float64) - r["acc2"].sum(dtype=np.float64)
    lam = np.float64(np.asarray(lambda_reg, dtype=np.float32))
    return np.asarray(np.float32(lam * total))
